# revision 31
# baseline (speedup 1.0000x reference)
"""Trainium2 Bass kernel for nn_Dumplicate_Removal (duplicate-removal attention).

Strategy (8 cores, 2 SPMD launches, no collectives):
  NEFF-1 (column-sharded): core c computes xT_c = relu(emb[rank] + W_vis.T@featT)
    [128, 256] (bf16, SBUF-only), then k/q/v PARTIAL products
    xT_c^T @ [Wk|Wq|Wv] col-slices -> kqvc [256, 2048] fp16 DMA'd out.
    Geometry weights gw for the core's 32-row block computed concurrently and
    fully overlapped with the feature streaming:
      - pair log-distance rows (3-way bf16 splits) are HOST-precomputed and
        shipped as the ready-to-matmul zrhs [14, 8192] operand; likewise the
        separable c2/c3 sin/cos factors (scq/p23) and the rank permutation
        matrix. This deletes the on-device Ln/abs/bounce front-end (~25
        dma_starts + an act-table switch) that serialized the z pipeline.
      - z = a (x) L outer product via K=14 bf16 matmuls, range reduction
        split across engines: chunks 0-3 round+frac on vector; chunks 4-7
        rnd on scalar (Identity+BIGF bias) and one fused vector op yielding
        -frac (sign folded into the contraction weights Cw),
      - Sin on scalar (fp16 out), single act table set for the whole NEFF,
      - contraction over the 128 (freq x cdim) rows via 32 selector matmuls
        into the gpre [32, 256] psum bank shared with the g23 openers.
  host: sums the 8 kqv fp16 partials in f32, adds biases, zeroes gw diagonals.
  NEFF-2 (row-sharded): vw = kT.q / sqrt(dk), att = exp(vw)*gw, row-normalize,
    feat = att @ v, sigmoid via exp+reciprocal (single act table set).
"""
import sys

for _p in ("/opt/trn_rl_repo", "/root/.axon_site/_ro/trn_rl_repo"):
    if _p not in sys.path:
        sys.path.append(_p)

import numpy as np
import concourse.bass as bass
import concourse.mybir as mybir
import concourse.tile as tile
from concourse import bacc
from concourse.bass_utils import run_bass_kernel_spmd
from concourse.masks import make_identity

F32 = mybir.dt.float32
BF16 = mybir.dt.bfloat16
FP16 = mybir.dt.float16
I32 = mybir.dt.int32
AT = mybir.ActivationFunctionType
OP = mybir.AluOpType

N = 256          # proposals
DHO = 4096       # feature dim
DMM = 1024       # model dim
DKEY = 512       # key dim
NCORES = 8
R = N // NCORES      # 32 rows per core (attention shard)
C = DMM // NCORES    # 128 mm-columns per core (fv shard)
M = 64               # frequencies
NKT = DHO // 128     # 32 contraction tiles for fv
TWO_PI = float(2 * np.pi)
BIGF = 12582912.0    # 1.5 * 2**23: (y + BIGF) - BIGF == round-to-nearest(y)
KZ = 14              # z-matmul contraction rows
ZCH = 1024           # z psum chunk (2 matmuls of 512, 2 psum banks)
NZC = R * N // ZCH   # 8 chunks
QD = NKT // 4        # 8 contraction tiles per fv quarter


def build_neff1():
    nc = bacc.Bacc("TRN2", target_bir_lowering=False, debug=False, num_devices=NCORES)
    featP = nc.dram_tensor("featP", [128, NKT * N], BF16, kind="ExternalInput")
    wvisP = nc.dram_tensor("wvisP", [128, NKT * C], BF16, kind="ExternalInput")
    embP = nc.dram_tensor("embP", [128, 2 * C], BF16, kind="ExternalInput")
    wkqvP = nc.dram_tensor("wkqvP", [128, 2048], BF16, kind="ExternalInput")
    zrhsP = nc.dram_tensor("zrhsP", [KZ, R * N], BF16, kind="ExternalInput")
    cwall = nc.dram_tensor("cwall", [128, R * R], FP16, kind="ExternalInput")
    abz = nc.dram_tensor("abz", [KZ, 128], BF16, kind="ExternalInput")
    scqp = nc.dram_tensor("scqp", [128, 2 * N + 2 * R], FP16, kind="ExternalInput")
    mpermP = nc.dram_tensor("mpermP", [128, 2 * N], BF16, kind="ExternalInput")
    colpk = nc.dram_tensor("colpk", [128, 2], F32, kind="ExternalInput")
    kqv_out = nc.dram_tensor("kqvc", [256, 2048], FP16, kind="ExternalOutput")
    gwc_out = nc.dram_tensor("gwc", [R, N], F32, kind="ExternalOutput")

    with tile.TileContext(nc) as tc:
        with (
            tc.tile_pool(name="const", bufs=1) as cpool,
            tc.tile_pool(name="work", bufs=2) as wpool,
            tc.tile_pool(name="big", bufs=1) as bpool,
            tc.tile_pool(name="psA", bufs=1, space="PSUM") as psA,   # fv: 1 bank
            tc.tile_pool(name="psZ", bufs=3, space="PSUM") as psZ,   # z: 6
            tc.tile_pool(name="psC", bufs=1, space="PSUM") as psC,   # gpre: 1
        ):
            # ---------- critical-path operands first, then streams ----------
            zrhs = bpool.tile([KZ, R * N], BF16, name="zrhs")
            nc.sync.dma_start(zrhs[:], zrhsP[:])
            abz_sb = cpool.tile([KZ, 128], BF16)
            nc.gpsimd.dma_start(abz_sb[:], abz[:])
            scqp_sb = cpool.tile([128, 2 * N + 2 * R], FP16)
            nc.scalar.dma_start(scqp_sb[:], scqp[:])
            bigcol = cpool.tile([128, 1], F32)
            nc.vector.memset(bigcol[:], BIGF)

            # stagger everything the z-pipeline does NOT need at start so the
            # zrhs/abz/scqp operands drain the HBM queues alone -- the z/sin
            # pipeline is engine-saturated, so starting earlier ends earlier
            with tc.tile_wait_until(0.008):
                cw_sb = cpool.tile([128, R * R], FP16)
                nc.sync.dma_start(cw_sb[:], cwall[:])
                colpk_sb = cpool.tile([128, 2], F32)
                nc.gpsimd.dma_start(colpk_sb[:], colpk[:])
                embt = cpool.tile([128, 2 * C], BF16)
                nc.gpsimd.dma_start(embt[:], embP[:])
                mperm = cpool.tile([128, 2 * N], BF16)
                nc.scalar.dma_start(mperm[:], mpermP[:])
                wkqv = cpool.tile([128, 2048], BF16)
                nc.scalar.dma_start(wkqv[:], wkqvP[:])

                fq_t = bpool.tile([128, NKT * N], BF16, name="fq_t")
                for qd, eng in enumerate((nc.sync, nc.gpsimd, nc.scalar, nc.sync)):
                    eng.dma_start(fq_t[:, qd * QD * N:(qd + 1) * QD * N],
                                  featP[:, qd * QD * N:(qd + 1) * QD * N])
                wv_t = bpool.tile([128, NKT * C], BF16, name="wv_t")
                for qd, eng in enumerate((nc.gpsimd, nc.sync, nc.scalar, nc.gpsimd)):
                    eng.dma_start(wv_t[:, qd * QD * C:(qd + 1) * QD * C],
                                  wvisP[:, qd * QD * C:(qd + 1) * QD * C])

            # ---------- z/sin pipeline + fv stream interleaved on PE ----------
            fvps = psA.tile([C, N], F32, name="fvps")
            s2 = bpool.tile([128, R * N], FP16, name="s2")
            gpre = psC.tile([R, N], F32, tag="pc", name="gpre")

            def fv_quarter(qd):
                # floor the modeled start at the real DMA landing time so the
                # static schedule doesn't order fv ahead of ready z/sin work
                with tc.tile_wait_until(0.018 + 0.0025 * qd):
                    for k2 in range(qd * QD, (qd + 1) * QD):
                        nc.tensor.matmul(fvps[:], wv_t[:, k2 * C:(k2 + 1) * C],
                                         fq_t[:, k2 * N:(k2 + 1) * N],
                                         start=(k2 == 0), stop=False)

            def z_chunk(chz):
                """chunks 0-3: vector rnd+frac (+frac); 4-7: scalar rnd +
                fused vector (-frac), sign folded into cwall columns."""
                zps = psZ.tile([128, ZCH], F32, tag="z", name=f"z{chz}")
                for h in range(ZCH // 512):
                    col0 = chz * ZCH + h * 512
                    nc.tensor.matmul(zps[:, h * 512:(h + 1) * 512], abz_sb[:],
                                     zrhs[:, col0:col0 + 512],
                                     start=True, stop=True)
                frac = wpool.tile([128, ZCH], F32, tag="frac", bufs=3)
                if chz < 4:
                    rnd = wpool.tile([128, ZCH], F32, tag="rnd", bufs=3)
                    nc.vector.tensor_scalar(rnd[:], zps[:], BIGF, -BIGF,
                                            OP.add, OP.add)
                    nc.vector.tensor_sub(frac[:], zps[:], rnd[:])
                else:
                    rndB = wpool.tile([128, ZCH], F32, tag="rnd", bufs=3)
                    nc.scalar.activation(rndB[:], zps[:], AT.Identity,
                                         bias=bigcol[:])
                    nc.vector.scalar_tensor_tensor(frac[:], rndB[:], -BIGF,
                                                   zps[:], OP.add, OP.subtract)
                nc.scalar.activation(s2[:, chz * ZCH:(chz + 1) * ZCH], frac[:],
                                     AT.Sin, scale=TWO_PI)

            def mv_chunk(chz):
                for ii in range(ZCH // N):
                    i = chz * (ZCH // N) + ii
                    nc.tensor.matmul(gpre[:], cw_sb[:, i * R:(i + 1) * R],
                                     s2[:, i * N:(i + 1) * N],
                                     start=False, stop=(i == R - 1))

            # gpre accumulation group opener (g23: separable c2/c3 features)
            nc.tensor.matmul(gpre[:], scqp_sb[:, 2 * N:2 * N + R],
                             scqp_sb[:, 0:N], start=True, stop=False)
            nc.tensor.matmul(gpre[:], scqp_sb[:, 2 * N + R:2 * N + 2 * R],
                             scqp_sb[:, N:2 * N], start=False, stop=False)
            z_chunk(0)
            z_chunk(1)
            z_chunk(2)
            z_chunk(3)
            z_chunk(4)
            z_chunk(5)
            z_chunk(6)
            z_chunk(7)
            fv_quarter(0)
            mv_chunk(0)
            mv_chunk(1)
            fv_quarter(1)
            mv_chunk(2)
            mv_chunk(3)
            fv_quarter(2)
            mv_chunk(4)
            mv_chunk(5)
            fv_quarter(3)
            for rb in range(2):
                nc.tensor.matmul(
                    fvps[:], embt[:, rb * C:(rb + 1) * C], mperm[:, rb * N:(rb + 1) * N],
                    start=False, stop=(rb == 1),
                )
            # keep the last mv chunks behind emb/xt/kqv in the static order --
            # they gate only the small gw output, not the kqv critical path
            with tc.tile_wait_until(0.036):
                mv_chunk(6)
                mv_chunk(7)

            gwt = cpool.tile([R, N], F32)
            nc.scalar.activation(gwt[:], gpre[:], AT.Relu,
                                 bias=colpk_sb[0:R, 1:2])
            nc.sync.dma_start(gwc_out[:], gwt[:])
            xt = cpool.tile([C, N], BF16)
            nc.scalar.activation(xt[:], fvps[:], AT.Relu, bias=colpk_sb[:, 0:1])

            # ---------- k/q/v partial products out (fp16) ----------
            kqvs = {}
            for half in range(2):
                kqvs[half] = wpool.tile([128, 2048], FP16, tag="kqvs",
                                        name=f"kqvs{half}")
                for ch in range(4):
                    idx = half * 4 + ch
                    pkv = psZ.tile([128, 512], F32, tag="z", name=f"pkv{idx}")
                    nc.tensor.matmul(pkv[:], xt[:, half * 128:(half + 1) * 128],
                                     wkqv[:, ch * 512:(ch + 1) * 512],
                                     start=True, stop=True)
                    dst = kqvs[half][:, ch * 512:(ch + 1) * 512]
                    if idx % 2 == 0:
                        nc.scalar.activation(dst, pkv[:], AT.Identity)
                    else:
                        nc.vector.tensor_copy(dst, pkv[:])
                nc.gpsimd.dma_start(
                    kqv_out[half * 128:(half + 1) * 128, :], kqvs[half][:])
    nc.compile()
    return nc


def build_neff2():
    nc = bacc.Bacc("TRN2", target_bir_lowering=False, debug=False, num_devices=NCORES)
    # kqvW pack: cols [0:4R] = kTl, [4R:4R+4N] = qT, [4R+4N:4R+4N+2DMM] = vW
    KQW = 4 * R + 4 * N + 2 * DMM
    kqvW = nc.dram_tensor("kqvW", [128, KQW], BF16, kind="ExternalInput")
    gwz = nc.dram_tensor("gwz", [R, N], F32, kind="ExternalInput")
    wrb_in = nc.dram_tensor("wrb", [R, DMM + 1], F32, kind="ExternalInput")
    outc = nc.dram_tensor("outc", [R, 1], F32, kind="ExternalOutput")

    with tile.TileContext(nc) as tc:
        with (
            tc.tile_pool(name="const", bufs=1) as cpool,
            tc.tile_pool(name="ps", bufs=1, space="PSUM") as psp,
            tc.tile_pool(name="pst", bufs=2, space="PSUM") as pst,
        ):
            KQ1 = 4 * R + 4 * N
            KQH = 4 * R + 2 * N
            kq = cpool.tile([128, KQW], BF16)
            # k-block plus first q half lands first: unblocks the first two
            # vw matmuls while the rest streams
            nc.sync.dma_start(kq[:, 0:KQH], kqvW[:, 0:KQH])
            nc.sync.dma_start(kq[:, KQH:KQ1], kqvW[:, KQH:KQ1])
            nc.scalar.dma_start(kq[:, KQ1:KQ1 + DMM], kqvW[:, KQ1:KQ1 + DMM])
            nc.gpsimd.dma_start(kq[:, KQ1 + DMM:], kqvW[:, KQ1 + DMM:])
            gw_t = cpool.tile([R, N], F32)
            nc.sync.dma_start(gw_t[:], gwz[:])
            wrb = cpool.tile([R, DMM + 1], F32)
            nc.gpsimd.dma_start(wrb[:], wrb_in[:])
            kt = kq[:, 0:4 * R]
            qt = kq[:, 4 * R:4 * R + 4 * N]
            vt = kq[:, 4 * R + 4 * N:]

            pvw = psp.tile([R, N], F32, name="pvw")
            for ob in range(4):
                nc.tensor.matmul(pvw[:], kt[:, ob * R:(ob + 1) * R],
                                 qt[:, ob * N:(ob + 1) * N],
                                 start=(ob == 0), stop=(ob == 3))
            e_t = cpool.tile([R, N], F32)
            nc.scalar.activation(e_t[:], pvw[:], AT.Exp,
                                 scale=float(1.0 / np.sqrt(DKEY)))
            att = cpool.tile([R, N], F32)
            nc.vector.tensor_mul(att[:], e_t[:], gw_t[:])
            rowsum = cpool.tile([R, 1], F32)
            nc.vector.reduce_sum(rowsum[:], att[:], axis=mybir.AxisListType.X)
            nc.vector.tensor_scalar(rowsum[:], rowsum[:], 1e-10, None, OP.add)
            recip = cpool.tile([R, 1], F32)
            nc.vector.reciprocal(recip[:], rowsum[:])

            # transpose the UNNORMALIZED att (row scale folded into zt later:
            # relu(feat/rs) == relu(feat)/rs for rs > 0) -- rowsum/recip drop
            # off the critical path
            ident = cpool.tile([128, 128], F32)
            make_identity(nc, ident[:])
            attT = cpool.tile([128, 2 * R], BF16)
            for jb in range(2):
                ptp = pst.tile([128, R], F32, tag="tp", name=f"ptp{jb}")
                nc.tensor.transpose(ptp[:], att[:, jb * 128:(jb + 1) * 128],
                                    ident[0:R, 0:R])
                nc.vector.tensor_copy(attT[:, jb * R:(jb + 1) * R], ptp[:])
            pf = psp.tile([R, DMM], F32, name="pf")
            for ch in range(2):
                for jb in range(2):
                    nc.tensor.matmul(pf[:, ch * 512:(ch + 1) * 512],
                                     attT[:, jb * R:(jb + 1) * R],
                                     vt[:, jb * DMM + ch * 512:jb * DMM + (ch + 1) * 512],
                                     start=(jb == 0), stop=(jb == 1))
            rl = cpool.tile([R, DMM], F32)
            nc.scalar.activation(rl[:], pf[:], AT.Relu)
            scr = cpool.tile([R, DMM], F32)
            nc.vector.tensor_mul(scr[:], rl[:], wrb[:, 0:DMM])
            zt = cpool.tile([R, 1], F32)
            nc.vector.reduce_sum(zt[:], scr[:], axis=mybir.AxisListType.X)
            ztn = cpool.tile([R, 1], F32)
            nc.vector.tensor_scalar(ztn[:], zt[:], recip[:], None, OP.mult)
            # sigmoid(ztn + br) = 1 / (1 + exp(-(ztn + br))) -- stays in the
            # exp table set (wrb col DMM holds -br)
            en = cpool.tile([R, 1], F32)
            nc.scalar.activation(en[:], ztn[:], AT.Exp, scale=-1.0,
                                 bias=wrb[:, DMM:DMM + 1])
            den = cpool.tile([R, 1], F32)
            nc.vector.tensor_scalar(den[:], en[:], 1.0, None, OP.add)
            ov = cpool.tile([R, 1], F32)
            nc.vector.reciprocal(ov[:], den[:])
            nc.sync.dma_start(outc[:], ov[:])
    nc.compile()
    return nc


_NC1 = None
_NC2 = None
TRACE = False
LAST_TIMES = []
LAST_RES = []


def _split3(x, bf):
    x1 = x.astype(bf).astype(np.float64)
    x2 = (x - x1).astype(bf).astype(np.float64)
    x3 = (x - x1 - x2).astype(bf).astype(np.float64)
    return x1, x2, x3


def kernel(feature_obj, highest_prob, rois_obj, emb_table, W_vis, b_vis,
           Wk, bk, Wq, bq, Wv, bv, Wg, bg, Wr, br):
    global _NC1, _NC2
    import ml_dtypes
    f32 = np.float32
    f64 = np.float64
    bf = ml_dtypes.bfloat16
    f16 = np.float16
    ca = np.ascontiguousarray

    featT = np.asarray(feature_obj, f32).T
    WvisT = np.asarray(W_vis, f32).T
    featP = ca(featT.reshape(NKT, 128, N).transpose(1, 0, 2)
               .reshape(128, NKT * N).astype(bf))
    # angles in revolutions
    alpha = (100.0 / (1000.0 ** (np.arange(M, dtype=f64) / M)) / (2 * np.pi))
    wg0 = np.asarray(Wg, f64)[0]
    hp = np.asarray(highest_prob, f32)

    # amplitude-phase fold for c0/c1: A sin(aL) + B cos(aL) = C sin(aL + phi)
    A01 = np.stack([wg0[0:64], wg0[128:192]])          # [cdim, m] sin coefs
    B01 = np.stack([wg0[64:128], wg0[192:256]])        # cos coefs
    Cmag = np.hypot(A01, B01)
    phi = np.arctan2(B01, A01) / (2 * np.pi)           # revolutions

    a1, a2_, a3 = _split3(alpha, bf)
    abz_m = np.zeros((KZ, 128))
    # pairing with zrhs rows (l1,l2,l3,l1,l2,l1): a1,a1,a1,a2,a2,a3
    for cdim in range(2):
        sl = slice(cdim * 64, (cdim + 1) * 64)
        for r, av in enumerate([a1, a1, a1, a2_, a2_, a3]):
            abz_m[cdim * 6 + r, sl] = av
    phi2 = np.concatenate([phi[0], phi[1]])
    p_hi = phi2.astype(bf).astype(f64)
    p_lo = (phi2 - p_hi).astype(bf).astype(f64)
    abz_m[12, :] = p_hi
    abz_m[13, :] = p_lo
    abz_m = ca(abz_m.astype(bf))

    Cw = np.concatenate([Cmag[0], Cmag[1]])
    cwall = np.zeros((128, R * R))
    for i in range(R):
        # z chunks 4-7 (rows 16-31) produce -sin: fold the sign into Cw
        sgn = 1.0 if i < 16 else -1.0
        cwall[:, i * R + i] = sgn * Cw
    cwall = ca(cwall.astype(f16))

    # ---- host geometry front-end ----
    rois = np.asarray(rois_obj, f64)
    x1, y1, x2, y2 = rois[:, 0], rois[:, 1], rois[:, 2], rois[:, 3]
    wv = x2 - x1 + 1e-10
    hv = y2 - y1 + 1e-10
    cxv = (x1 + x2) / 2
    cyv = (y1 + y2) / 2
    lw = np.log(wv)
    lh = np.log(hv)

    # scq: sin/cos of a*log(w_j), a*log(h_j) over all proposals (shared)
    alpha2 = np.concatenate([alpha, alpha])
    offq = np.concatenate([np.full(M, 0.25), np.zeros(M)])
    lwhflat = np.concatenate([lw, lh])
    scq_h = np.sin(TWO_PI * (alpha2[:, None] * lwhflat[None, :]
                             + offq[:, None])).astype(f16)      # [128, 2N]

    # mperm: permutation matrix from the stable descending argsort of hp
    rank = np.argsort(-hp, kind="stable")
    mperm_h = np.zeros((128, 2 * N), f32)
    for j in range(N):
        r_ = int(rank[j])
        mperm_h[r_ % 128, (r_ // 128) * N + j] = 1.0
    mperm_h = ca(mperm_h.astype(bf))

    wkT = np.asarray(Wk, f32).T     # [1024, 512]
    wqT = np.asarray(Wq, f32).T
    wvT = np.asarray(Wv, f32).T     # [1024, 1024]
    wkqv_all = np.concatenate([wkT, wqT, wvT], axis=1)  # [1024, 2048]

    if _NC1 is None:
        _NC1 = build_neff1()
    in1 = []
    for c in range(NCORES):
        wvisPc = ca(WvisT[:, c * C:(c + 1) * C].reshape(NKT, 128, C)
                    .transpose(1, 0, 2).reshape(128, NKT * C).astype(bf))
        embPc = ca(np.asarray(emb_table, f32)[:, c * C:(c + 1) * C]
                   .reshape(2, 128, C).transpose(1, 0, 2)
                   .reshape(128, 2 * C).astype(bf))
        # zrhs: pair log-distance rows for this core's 32 keys
        sl = slice(c * R, (c + 1) * R)
        zr = np.zeros((KZ, R, N))
        for cdim, (cv, wl, lg) in enumerate(((cxv, wv, lw), (cyv, hv, lh))):
            d = np.abs(cv[None, :] - cv[sl, None])              # [R, N]
            with np.errstate(divide="ignore"):
                L = np.where(d == 0, 0.0, np.log(d) - lg[sl, None])
            l1_, l2_, l3_ = _split3(L, bf)
            for r_, lsp in enumerate((l1_, l2_, l3_, l1_, l2_, l1_)):
                zr[cdim * 6 + r_] = lsp
        zr[12] = 1.0
        zr[13] = 1.0
        zrhs_c = ca(zr.reshape(KZ, R * N).astype(bf))

        # p23: per-key separable factors for c2 (w ratio) and c3 (h ratio)
        scqp_c = np.zeros((128, 2 * N + 2 * R))
        scqp_c[:, 0:2 * N] = scq_h.astype(f64)
        for cdim, lg in ((2, lw), (3, lh)):
            A_ = wg0[cdim * 128:cdim * 128 + 64]
            B_ = wg0[cdim * 128 + 64:cdim * 128 + 128]
            th = TWO_PI * alpha[:, None] * lg[sl][None, :]      # [64, R]
            s_, c_ = np.sin(th), np.cos(th)
            col0 = 2 * N + (cdim - 2) * R
            scqp_c[0:64, col0:col0 + R] = A_[:, None] * s_ + B_[:, None] * c_
            scqp_c[64:128, col0:col0 + R] = B_[:, None] * s_ - A_[:, None] * c_
        scqp_c = ca(scqp_c.astype(f16))

        colpk_c = np.zeros((128, 2), f32)
        colpk_c[:, 0] = np.asarray(b_vis, f32)[c * C:(c + 1) * C]
        colpk_c[:, 1] = float(np.asarray(bg, f32)[0])
        in1.append(dict(
            featP=featP,
            wvisP=wvisPc,
            embP=embPc,
            wkqvP=ca(wkqv_all[c * C:(c + 1) * C, :].astype(bf)),
            zrhsP=zrhs_c,
            cwall=cwall,
            abz=abz_m,
            scqp=scqp_c,
            mpermP=mperm_h,
            colpk=colpk_c,
        ))
    res1 = run_bass_kernel_spmd(_NC1, in1, list(range(NCORES)), trace=TRACE)
    if TRACE:
        LAST_TIMES.append(res1.exec_time_ns)
        LAST_RES.append(res1)

    # host: sum kqv partials, add biases, split k/q/v
    acc = np.zeros((256, 2048), f32)
    for c in range(NCORES):
        acc += res1.results[c]["kqvc"].astype(f32)
    k_full = acc[:, 0:512] + np.asarray(bk, f32)
    q_full = acc[:, 512:1024] + np.asarray(bq, f32)
    v_full = acc[:, 1024:2048] + np.asarray(bv, f32)
    gws = [res1.results[c]["gwc"].copy() for c in range(NCORES)]
    for c in range(NCORES):
        for i in range(R):
            gws[c][i, c * R + i] = 0.0

    if _NC2 is None:
        _NC2 = build_neff2()

    qTp = q_full.T.reshape(4, 128, N).transpose(1, 0, 2).reshape(128, 4 * N)
    vWp = v_full.reshape(2, 128, DMM).transpose(1, 0, 2).reshape(128, 2 * DMM)
    wrb_h = np.zeros((R, DMM + 1), f32)
    wrb_h[:, 0:DMM] = np.asarray(Wr, f32)[0]
    wrb_h[:, DMM] = -float(np.asarray(br, f32)[0])
    wrb_h = ca(wrb_h)
    in2 = []
    for c in range(NCORES):
        kl = k_full[c * R:(c + 1) * R, :]           # [32, 512]
        kTlp = kl.T.reshape(4, 128, R).transpose(1, 0, 2).reshape(128, 4 * R)
        kqvW_c = ca(np.concatenate([kTlp, qTp, vWp], axis=1).astype(bf))
        in2.append(dict(
            kqvW=kqvW_c,
            gwz=gws[c],
            wrb=wrb_h,
        ))
    res2 = run_bass_kernel_spmd(_NC2, in2, list(range(NCORES)), trace=TRACE)
    if TRACE:
        LAST_TIMES.append(res2.exec_time_ns)
        LAST_RES.append(res2)
    out = np.concatenate([res2.results[c]["outc"] for c in range(NCORES)], axis=0)
    return out.astype(f32)


# revision 32
# speedup vs baseline: 1.0085x; 1.0085x over previous
"""Trainium2 Bass kernel for nn_Dumplicate_Removal (duplicate-removal attention).

Strategy (8 cores, 2 SPMD launches, no collectives):
  NEFF-1 (column-sharded): core c computes xT_c = relu(emb[rank] + W_vis.T@featT)
    [128, 256] (bf16, SBUF-only), then k/q/v PARTIAL products
    xT_c^T @ [Wk|Wq|Wv] col-slices -> kqvc [256, 2048] fp16 DMA'd out.
    Geometry weights gw for the core's 32-row block computed concurrently and
    fully overlapped with the feature streaming:
      - pair log-distance rows (3-way bf16 splits) are HOST-precomputed and
        shipped as the ready-to-matmul zrhs [14, 8192] operand; likewise the
        separable c2/c3 sin/cos factors (scq/p23) and the rank permutation
        matrix. This deletes the on-device Ln/abs/bounce front-end (~25
        dma_starts + an act-table switch) that serialized the z pipeline.
      - z = a (x) L outer product via K=14 bf16 matmuls, range reduction
        split across engines: chunks 0-3 round+frac on vector; chunks 4-7
        rnd on scalar (Identity+BIGF bias) and one fused vector op yielding
        -frac (sign folded into the contraction weights Cw),
      - Sin on scalar (fp16 out), single act table set for the whole NEFF,
      - contraction over the 128 (freq x cdim) rows via 32 selector matmuls
        into the gpre [32, 256] psum bank shared with the g23 openers.
  host: sums the 8 kqv fp16 partials in f32, adds biases, zeroes gw diagonals.
  NEFF-2 (row-sharded): vw = kT.q / sqrt(dk), att = exp(vw)*gw, row-normalize,
    feat = att @ v, sigmoid via exp+reciprocal (single act table set).
"""
import sys

for _p in ("/opt/trn_rl_repo", "/root/.axon_site/_ro/trn_rl_repo"):
    if _p not in sys.path:
        sys.path.append(_p)

import numpy as np
import concourse.bass as bass
import concourse.mybir as mybir
import concourse.tile as tile
from concourse import bacc
from concourse.bass_utils import run_bass_kernel_spmd
from concourse.masks import make_identity

F32 = mybir.dt.float32
BF16 = mybir.dt.bfloat16
FP16 = mybir.dt.float16
I32 = mybir.dt.int32
AT = mybir.ActivationFunctionType
OP = mybir.AluOpType

N = 256          # proposals
DHO = 4096       # feature dim
DMM = 1024       # model dim
DKEY = 512       # key dim
NCORES = 8
R = N // NCORES      # 32 rows per core (attention shard)
C = DMM // NCORES    # 128 mm-columns per core (fv shard)
M = 64               # frequencies
NKT = DHO // 128     # 32 contraction tiles for fv
TWO_PI = float(2 * np.pi)
BIGF = 12582912.0    # 1.5 * 2**23: (y + BIGF) - BIGF == round-to-nearest(y)
KZ = 14              # z-matmul contraction rows
ZCH = 1024           # z psum chunk (2 matmuls of 512, 2 psum banks)
NZC = R * N // ZCH   # 8 chunks
QD = NKT // 4        # 8 contraction tiles per fv quarter


def build_neff1():
    nc = bacc.Bacc("TRN2", target_bir_lowering=False, debug=False, num_devices=NCORES)
    featP = nc.dram_tensor("featP", [128, NKT * N], BF16, kind="ExternalInput")
    wvisP = nc.dram_tensor("wvisP", [128, NKT * C], BF16, kind="ExternalInput")
    embP = nc.dram_tensor("embP", [128, 2 * C], BF16, kind="ExternalInput")
    wkqvP = nc.dram_tensor("wkqvP", [128, 2048], BF16, kind="ExternalInput")
    zrhsP = nc.dram_tensor("zrhsP", [KZ, R * N], BF16, kind="ExternalInput")
    cwall = nc.dram_tensor("cwall", [128, R * R], FP16, kind="ExternalInput")
    abz = nc.dram_tensor("abz", [KZ, 128], BF16, kind="ExternalInput")
    scqp = nc.dram_tensor("scqp", [128, 2 * N + 2 * R], FP16, kind="ExternalInput")
    mpermP = nc.dram_tensor("mpermP", [128, 2 * N], BF16, kind="ExternalInput")
    colpk = nc.dram_tensor("colpk", [128, 2], F32, kind="ExternalInput")
    kqv_out = nc.dram_tensor("kqvc", [256, 2048], FP16, kind="ExternalOutput")
    gwc_out = nc.dram_tensor("gwc", [R, N], F32, kind="ExternalOutput")

    with tile.TileContext(nc) as tc:
        with (
            tc.tile_pool(name="const", bufs=1) as cpool,
            tc.tile_pool(name="work", bufs=2) as wpool,
            tc.tile_pool(name="big", bufs=1) as bpool,
            tc.tile_pool(name="psA", bufs=1, space="PSUM") as psA,   # fv: 1 bank
            tc.tile_pool(name="psZ", bufs=3, space="PSUM") as psZ,   # z: 6
            tc.tile_pool(name="psC", bufs=1, space="PSUM") as psC,   # gpre: 1
        ):
            # ---------- critical-path operands first, then streams ----------
            zrhs = bpool.tile([KZ, R * N], BF16, name="zrhs")
            nc.sync.dma_start(zrhs[:], zrhsP[:])
            abz_sb = cpool.tile([KZ, 128], BF16)
            nc.gpsimd.dma_start(abz_sb[:], abz[:])
            scqp_sb = cpool.tile([128, 2 * N + 2 * R], FP16)
            nc.scalar.dma_start(scqp_sb[:], scqp[:])
            bigcol = cpool.tile([128, 1], F32)
            nc.vector.memset(bigcol[:], BIGF)

            # stagger everything the z-pipeline does NOT need at start so the
            # zrhs/abz/scqp operands drain the HBM queues alone -- the z/sin
            # pipeline is engine-saturated, so starting earlier ends earlier
            with tc.tile_wait_until(0.008):
                cw_sb = cpool.tile([128, R * R], FP16)
                nc.sync.dma_start(cw_sb[:], cwall[:])
                colpk_sb = cpool.tile([128, 2], F32)
                nc.gpsimd.dma_start(colpk_sb[:], colpk[:])
                embt = cpool.tile([128, 2 * C], BF16)
                nc.gpsimd.dma_start(embt[:], embP[:])
                mperm = cpool.tile([128, 2 * N], BF16)
                nc.scalar.dma_start(mperm[:], mpermP[:])
                wkqv = cpool.tile([128, 2048], BF16)
                nc.scalar.dma_start(wkqv[:], wkqvP[:])

                fq_t = bpool.tile([128, NKT * N], BF16, name="fq_t")
                for qd, eng in enumerate((nc.sync, nc.gpsimd, nc.scalar, nc.sync)):
                    eng.dma_start(fq_t[:, qd * QD * N:(qd + 1) * QD * N],
                                  featP[:, qd * QD * N:(qd + 1) * QD * N])
                wv_t = bpool.tile([128, NKT * C], BF16, name="wv_t")
                for qd, eng in enumerate((nc.gpsimd, nc.sync, nc.scalar, nc.gpsimd)):
                    eng.dma_start(wv_t[:, qd * QD * C:(qd + 1) * QD * C],
                                  wvisP[:, qd * QD * C:(qd + 1) * QD * C])

            # ---------- z/sin pipeline + fv stream interleaved on PE ----------
            fvps = psA.tile([C, N], F32, name="fvps")
            s2 = bpool.tile([128, R * N], FP16, name="s2")
            gpre = psC.tile([R, N], F32, tag="pc", name="gpre")

            def fv_quarter(qd):
                # floor the modeled start at the real DMA landing time so the
                # static schedule doesn't order fv ahead of ready z/sin work
                with tc.tile_wait_until(0.018 + 0.0025 * qd):
                    for k2 in range(qd * QD, (qd + 1) * QD):
                        nc.tensor.matmul(fvps[:], wv_t[:, k2 * C:(k2 + 1) * C],
                                         fq_t[:, k2 * N:(k2 + 1) * N],
                                         start=(k2 == 0), stop=False)

            def z_chunk(chz):
                """chunks 0-3: vector rnd+frac (+frac); 4-7: scalar rnd +
                fused vector (-frac), sign folded into cwall columns."""
                zps = psZ.tile([128, ZCH], F32, tag="z", name=f"z{chz}")
                for h in range(ZCH // 512):
                    col0 = chz * ZCH + h * 512
                    nc.tensor.matmul(zps[:, h * 512:(h + 1) * 512], abz_sb[:],
                                     zrhs[:, col0:col0 + 512],
                                     start=True, stop=True)
                frac = wpool.tile([128, ZCH], F32, tag="frac")
                if chz < 4:
                    rnd = wpool.tile([128, ZCH], F32, tag="rnd")
                    nc.vector.tensor_scalar(rnd[:], zps[:], BIGF, -BIGF,
                                            OP.add, OP.add)
                    nc.vector.tensor_sub(frac[:], zps[:], rnd[:])
                else:
                    rndB = wpool.tile([128, ZCH], F32, tag="rnd")
                    nc.scalar.activation(rndB[:], zps[:], AT.Identity,
                                         bias=bigcol[:])
                    nc.vector.scalar_tensor_tensor(frac[:], rndB[:], -BIGF,
                                                   zps[:], OP.add, OP.subtract)
                nc.scalar.activation(s2[:, chz * ZCH:(chz + 1) * ZCH], frac[:],
                                     AT.Sin, scale=TWO_PI)

            def mv_chunk(chz):
                for ii in range(ZCH // N):
                    i = chz * (ZCH // N) + ii
                    nc.tensor.matmul(gpre[:], cw_sb[:, i * R:(i + 1) * R],
                                     s2[:, i * N:(i + 1) * N],
                                     start=False, stop=(i == R - 1))

            # gpre accumulation group opener (g23: separable c2/c3 features)
            nc.tensor.matmul(gpre[:], scqp_sb[:, 2 * N:2 * N + R],
                             scqp_sb[:, 0:N], start=True, stop=False)
            nc.tensor.matmul(gpre[:], scqp_sb[:, 2 * N + R:2 * N + 2 * R],
                             scqp_sb[:, N:2 * N], start=False, stop=False)
            z_chunk(0)
            z_chunk(1)
            z_chunk(2)
            z_chunk(3)
            z_chunk(4)
            z_chunk(5)
            z_chunk(6)
            z_chunk(7)
            fv_quarter(0)
            mv_chunk(0)
            mv_chunk(1)
            fv_quarter(1)
            mv_chunk(2)
            mv_chunk(3)
            fv_quarter(2)
            mv_chunk(4)
            mv_chunk(5)
            fv_quarter(3)
            for rb in range(2):
                nc.tensor.matmul(
                    fvps[:], embt[:, rb * C:(rb + 1) * C], mperm[:, rb * N:(rb + 1) * N],
                    start=False, stop=(rb == 1),
                )
            # keep the last mv chunks behind emb/xt/kqv in the static order --
            # they gate only the small gw output, not the kqv critical path
            with tc.tile_wait_until(0.036):
                mv_chunk(6)
                mv_chunk(7)

            gwt = cpool.tile([R, N], F32)
            nc.scalar.activation(gwt[:], gpre[:], AT.Relu,
                                 bias=colpk_sb[0:R, 1:2])
            nc.sync.dma_start(gwc_out[:], gwt[:])
            xt = cpool.tile([C, N], BF16)
            nc.scalar.activation(xt[:], fvps[:], AT.Relu, bias=colpk_sb[:, 0:1])

            # ---------- k/q/v partial products out (fp16) ----------
            kqvs = {}
            for half in range(2):
                kqvs[half] = wpool.tile([128, 2048], FP16, tag="kqvs",
                                        name=f"kqvs{half}")
                for ch in range(4):
                    idx = half * 4 + ch
                    pkv = psZ.tile([128, 512], F32, tag="z", name=f"pkv{idx}")
                    nc.tensor.matmul(pkv[:], xt[:, half * 128:(half + 1) * 128],
                                     wkqv[:, ch * 512:(ch + 1) * 512],
                                     start=True, stop=True)
                    dst = kqvs[half][:, ch * 512:(ch + 1) * 512]
                    if idx % 2 == 0:
                        nc.scalar.activation(dst, pkv[:], AT.Identity)
                    else:
                        nc.vector.tensor_copy(dst, pkv[:])
                nc.gpsimd.dma_start(
                    kqv_out[half * 128:(half + 1) * 128, :], kqvs[half][:])
    nc.compile()
    return nc


def build_neff2():
    nc = bacc.Bacc("TRN2", target_bir_lowering=False, debug=False, num_devices=NCORES)
    # kqvW pack: cols [0:4R] = kTl, [4R:4R+4N] = qT, [4R+4N:4R+4N+2DMM] = vW
    KQW = 4 * R + 4 * N + 2 * DMM
    kqvW = nc.dram_tensor("kqvW", [128, KQW], BF16, kind="ExternalInput")
    gwz = nc.dram_tensor("gwz", [R, N], F32, kind="ExternalInput")
    wrb_in = nc.dram_tensor("wrb", [R, DMM + 1], F32, kind="ExternalInput")
    outc = nc.dram_tensor("outc", [R, 1], F32, kind="ExternalOutput")

    with tile.TileContext(nc) as tc:
        with (
            tc.tile_pool(name="const", bufs=1) as cpool,
            tc.tile_pool(name="ps", bufs=1, space="PSUM") as psp,
            tc.tile_pool(name="pst", bufs=2, space="PSUM") as pst,
        ):
            KQ1 = 4 * R + 4 * N
            KQH = 4 * R + 2 * N
            kq = cpool.tile([128, KQW], BF16)
            # k-block plus first q half lands first: unblocks the first two
            # vw matmuls while the rest streams
            nc.sync.dma_start(kq[:, 0:KQH], kqvW[:, 0:KQH])
            nc.sync.dma_start(kq[:, KQH:KQ1], kqvW[:, KQH:KQ1])
            nc.scalar.dma_start(kq[:, KQ1:KQ1 + DMM], kqvW[:, KQ1:KQ1 + DMM])
            nc.gpsimd.dma_start(kq[:, KQ1 + DMM:], kqvW[:, KQ1 + DMM:])
            gw_t = cpool.tile([R, N], F32)
            nc.sync.dma_start(gw_t[:], gwz[:])
            wrb = cpool.tile([R, DMM + 1], F32)
            nc.gpsimd.dma_start(wrb[:], wrb_in[:])
            kt = kq[:, 0:4 * R]
            qt = kq[:, 4 * R:4 * R + 4 * N]
            vt = kq[:, 4 * R + 4 * N:]

            pvw = psp.tile([R, N], F32, name="pvw")
            for ob in range(4):
                nc.tensor.matmul(pvw[:], kt[:, ob * R:(ob + 1) * R],
                                 qt[:, ob * N:(ob + 1) * N],
                                 start=(ob == 0), stop=(ob == 3))
            e_t = cpool.tile([R, N], F32)
            nc.scalar.activation(e_t[:], pvw[:], AT.Exp,
                                 scale=float(1.0 / np.sqrt(DKEY)))
            att = cpool.tile([R, N], F32)
            nc.vector.tensor_mul(att[:], e_t[:], gw_t[:])
            rowsum = cpool.tile([R, 1], F32)
            nc.vector.reduce_sum(rowsum[:], att[:], axis=mybir.AxisListType.X)
            nc.vector.tensor_scalar(rowsum[:], rowsum[:], 1e-10, None, OP.add)
            recip = cpool.tile([R, 1], F32)
            nc.vector.reciprocal(recip[:], rowsum[:])

            # transpose the UNNORMALIZED att (row scale folded into zt later:
            # relu(feat/rs) == relu(feat)/rs for rs > 0) -- rowsum/recip drop
            # off the critical path
            ident = cpool.tile([128, 128], F32)
            make_identity(nc, ident[:])
            attT = cpool.tile([128, 2 * R], BF16)
            for jb in range(2):
                ptp = pst.tile([128, R], F32, tag="tp", name=f"ptp{jb}")
                nc.tensor.transpose(ptp[:], att[:, jb * 128:(jb + 1) * 128],
                                    ident[0:R, 0:R])
                nc.vector.tensor_copy(attT[:, jb * R:(jb + 1) * R], ptp[:])
            pf = psp.tile([R, DMM], F32, name="pf")
            for ch in range(2):
                for jb in range(2):
                    nc.tensor.matmul(pf[:, ch * 512:(ch + 1) * 512],
                                     attT[:, jb * R:(jb + 1) * R],
                                     vt[:, jb * DMM + ch * 512:jb * DMM + (ch + 1) * 512],
                                     start=(jb == 0), stop=(jb == 1))
            rl = cpool.tile([R, DMM], F32)
            nc.scalar.activation(rl[:], pf[:], AT.Relu)
            scr = cpool.tile([R, DMM], F32)
            nc.vector.tensor_mul(scr[:], rl[:], wrb[:, 0:DMM])
            zt = cpool.tile([R, 1], F32)
            nc.vector.reduce_sum(zt[:], scr[:], axis=mybir.AxisListType.X)
            ztn = cpool.tile([R, 1], F32)
            nc.vector.tensor_scalar(ztn[:], zt[:], recip[:], None, OP.mult)
            # sigmoid(ztn + br) = 1 / (1 + exp(-(ztn + br))) -- stays in the
            # exp table set (wrb col DMM holds -br)
            en = cpool.tile([R, 1], F32)
            nc.scalar.activation(en[:], ztn[:], AT.Exp, scale=-1.0,
                                 bias=wrb[:, DMM:DMM + 1])
            den = cpool.tile([R, 1], F32)
            nc.vector.tensor_scalar(den[:], en[:], 1.0, None, OP.add)
            ov = cpool.tile([R, 1], F32)
            nc.vector.reciprocal(ov[:], den[:])
            nc.sync.dma_start(outc[:], ov[:])
    nc.compile()
    return nc


_NC1 = None
_NC2 = None
TRACE = False
LAST_TIMES = []
LAST_RES = []


def _split3(x, bf):
    x1 = x.astype(bf).astype(np.float64)
    x2 = (x - x1).astype(bf).astype(np.float64)
    x3 = (x - x1 - x2).astype(bf).astype(np.float64)
    return x1, x2, x3


def kernel(feature_obj, highest_prob, rois_obj, emb_table, W_vis, b_vis,
           Wk, bk, Wq, bq, Wv, bv, Wg, bg, Wr, br):
    global _NC1, _NC2
    import ml_dtypes
    f32 = np.float32
    f64 = np.float64
    bf = ml_dtypes.bfloat16
    f16 = np.float16
    ca = np.ascontiguousarray

    featT = np.asarray(feature_obj, f32).T
    WvisT = np.asarray(W_vis, f32).T
    featP = ca(featT.reshape(NKT, 128, N).transpose(1, 0, 2)
               .reshape(128, NKT * N).astype(bf))
    # angles in revolutions
    alpha = (100.0 / (1000.0 ** (np.arange(M, dtype=f64) / M)) / (2 * np.pi))
    wg0 = np.asarray(Wg, f64)[0]
    hp = np.asarray(highest_prob, f32)

    # amplitude-phase fold for c0/c1: A sin(aL) + B cos(aL) = C sin(aL + phi)
    A01 = np.stack([wg0[0:64], wg0[128:192]])          # [cdim, m] sin coefs
    B01 = np.stack([wg0[64:128], wg0[192:256]])        # cos coefs
    Cmag = np.hypot(A01, B01)
    phi = np.arctan2(B01, A01) / (2 * np.pi)           # revolutions

    a1, a2_, a3 = _split3(alpha, bf)
    abz_m = np.zeros((KZ, 128))
    # pairing with zrhs rows (l1,l2,l3,l1,l2,l1): a1,a1,a1,a2,a2,a3
    for cdim in range(2):
        sl = slice(cdim * 64, (cdim + 1) * 64)
        for r, av in enumerate([a1, a1, a1, a2_, a2_, a3]):
            abz_m[cdim * 6 + r, sl] = av
    phi2 = np.concatenate([phi[0], phi[1]])
    p_hi = phi2.astype(bf).astype(f64)
    p_lo = (phi2 - p_hi).astype(bf).astype(f64)
    abz_m[12, :] = p_hi
    abz_m[13, :] = p_lo
    abz_m = ca(abz_m.astype(bf))

    Cw = np.concatenate([Cmag[0], Cmag[1]])
    cwall = np.zeros((128, R * R))
    for i in range(R):
        # z chunks 4-7 (rows 16-31) produce -sin: fold the sign into Cw
        sgn = 1.0 if i < 16 else -1.0
        cwall[:, i * R + i] = sgn * Cw
    cwall = ca(cwall.astype(f16))

    # ---- host geometry front-end ----
    rois = np.asarray(rois_obj, f64)
    x1, y1, x2, y2 = rois[:, 0], rois[:, 1], rois[:, 2], rois[:, 3]
    wv = x2 - x1 + 1e-10
    hv = y2 - y1 + 1e-10
    cxv = (x1 + x2) / 2
    cyv = (y1 + y2) / 2
    lw = np.log(wv)
    lh = np.log(hv)

    # scq: sin/cos of a*log(w_j), a*log(h_j) over all proposals (shared)
    alpha2 = np.concatenate([alpha, alpha])
    offq = np.concatenate([np.full(M, 0.25), np.zeros(M)])
    lwhflat = np.concatenate([lw, lh])
    scq_h = np.sin(TWO_PI * (alpha2[:, None] * lwhflat[None, :]
                             + offq[:, None])).astype(f16)      # [128, 2N]

    # mperm: permutation matrix from the stable descending argsort of hp
    rank = np.argsort(-hp, kind="stable")
    mperm_h = np.zeros((128, 2 * N), f32)
    for j in range(N):
        r_ = int(rank[j])
        mperm_h[r_ % 128, (r_ // 128) * N + j] = 1.0
    mperm_h = ca(mperm_h.astype(bf))

    wkT = np.asarray(Wk, f32).T     # [1024, 512]
    wqT = np.asarray(Wq, f32).T
    wvT = np.asarray(Wv, f32).T     # [1024, 1024]
    wkqv_all = np.concatenate([wkT, wqT, wvT], axis=1)  # [1024, 2048]

    if _NC1 is None:
        _NC1 = build_neff1()
    in1 = []
    for c in range(NCORES):
        wvisPc = ca(WvisT[:, c * C:(c + 1) * C].reshape(NKT, 128, C)
                    .transpose(1, 0, 2).reshape(128, NKT * C).astype(bf))
        embPc = ca(np.asarray(emb_table, f32)[:, c * C:(c + 1) * C]
                   .reshape(2, 128, C).transpose(1, 0, 2)
                   .reshape(128, 2 * C).astype(bf))
        # zrhs: pair log-distance rows for this core's 32 keys
        sl = slice(c * R, (c + 1) * R)
        zr = np.zeros((KZ, R, N))
        for cdim, (cv, wl, lg) in enumerate(((cxv, wv, lw), (cyv, hv, lh))):
            d = np.abs(cv[None, :] - cv[sl, None])              # [R, N]
            with np.errstate(divide="ignore"):
                L = np.where(d == 0, 0.0, np.log(d) - lg[sl, None])
            l1_, l2_, l3_ = _split3(L, bf)
            for r_, lsp in enumerate((l1_, l2_, l3_, l1_, l2_, l1_)):
                zr[cdim * 6 + r_] = lsp
        zr[12] = 1.0
        zr[13] = 1.0
        zrhs_c = ca(zr.reshape(KZ, R * N).astype(bf))

        # p23: per-key separable factors for c2 (w ratio) and c3 (h ratio)
        scqp_c = np.zeros((128, 2 * N + 2 * R))
        scqp_c[:, 0:2 * N] = scq_h.astype(f64)
        for cdim, lg in ((2, lw), (3, lh)):
            A_ = wg0[cdim * 128:cdim * 128 + 64]
            B_ = wg0[cdim * 128 + 64:cdim * 128 + 128]
            th = TWO_PI * alpha[:, None] * lg[sl][None, :]      # [64, R]
            s_, c_ = np.sin(th), np.cos(th)
            col0 = 2 * N + (cdim - 2) * R
            scqp_c[0:64, col0:col0 + R] = A_[:, None] * s_ + B_[:, None] * c_
            scqp_c[64:128, col0:col0 + R] = B_[:, None] * s_ - A_[:, None] * c_
        scqp_c = ca(scqp_c.astype(f16))

        colpk_c = np.zeros((128, 2), f32)
        colpk_c[:, 0] = np.asarray(b_vis, f32)[c * C:(c + 1) * C]
        colpk_c[:, 1] = float(np.asarray(bg, f32)[0])
        in1.append(dict(
            featP=featP,
            wvisP=wvisPc,
            embP=embPc,
            wkqvP=ca(wkqv_all[c * C:(c + 1) * C, :].astype(bf)),
            zrhsP=zrhs_c,
            cwall=cwall,
            abz=abz_m,
            scqp=scqp_c,
            mpermP=mperm_h,
            colpk=colpk_c,
        ))
    res1 = run_bass_kernel_spmd(_NC1, in1, list(range(NCORES)), trace=TRACE)
    if TRACE:
        LAST_TIMES.append(res1.exec_time_ns)
        LAST_RES.append(res1)

    # host: sum kqv partials, add biases, split k/q/v
    acc = np.zeros((256, 2048), f32)
    for c in range(NCORES):
        acc += res1.results[c]["kqvc"].astype(f32)
    k_full = acc[:, 0:512] + np.asarray(bk, f32)
    q_full = acc[:, 512:1024] + np.asarray(bq, f32)
    v_full = acc[:, 1024:2048] + np.asarray(bv, f32)
    gws = [res1.results[c]["gwc"].copy() for c in range(NCORES)]
    for c in range(NCORES):
        for i in range(R):
            gws[c][i, c * R + i] = 0.0

    if _NC2 is None:
        _NC2 = build_neff2()

    qTp = q_full.T.reshape(4, 128, N).transpose(1, 0, 2).reshape(128, 4 * N)
    vWp = v_full.reshape(2, 128, DMM).transpose(1, 0, 2).reshape(128, 2 * DMM)
    wrb_h = np.zeros((R, DMM + 1), f32)
    wrb_h[:, 0:DMM] = np.asarray(Wr, f32)[0]
    wrb_h[:, DMM] = -float(np.asarray(br, f32)[0])
    wrb_h = ca(wrb_h)
    in2 = []
    for c in range(NCORES):
        kl = k_full[c * R:(c + 1) * R, :]           # [32, 512]
        kTlp = kl.T.reshape(4, 128, R).transpose(1, 0, 2).reshape(128, 4 * R)
        kqvW_c = ca(np.concatenate([kTlp, qTp, vWp], axis=1).astype(bf))
        in2.append(dict(
            kqvW=kqvW_c,
            gwz=gws[c],
            wrb=wrb_h,
        ))
    res2 = run_bass_kernel_spmd(_NC2, in2, list(range(NCORES)), trace=TRACE)
    if TRACE:
        LAST_TIMES.append(res2.exec_time_ns)
        LAST_RES.append(res2)
    out = np.concatenate([res2.results[c]["outc"] for c in range(NCORES)], axis=0)
    return out.astype(f32)
